# revision 1
# baseline (speedup 1.0000x reference)
"""ArcFace (AngularPenaltySMLoss) on 8 TRN2 NeuronCores.

Strategy: data-parallel over batch rows. pred is [1024, 100000] f32; each of
the 8 cores gets a [128, 100000] shard and computes, per row, the full-row
sum of exp(64 * pred) in a single streaming pass (DMA -> ScalarEngine
activation Exp with fused per-partition accumulation). The tiny epilogue
(label gather, arccos/cos numerator, log, mean) is O(B) and runs on host.
"""

import sys

import numpy as np

_REPO = "/opt/trn_rl_repo"
if _REPO not in sys.path:
    sys.path.insert(0, _REPO)

import concourse.bass as bass
import concourse.tile as tile
from concourse import bacc, mybir
from concourse.bass_utils import run_bass_kernel_spmd

B, C = 1024, 100000
N_CORES = 8
ROWS = B // N_CORES  # 128 rows per core = SBUF partition count
F = 6250             # columns per SBUF tile (128 x 6250 f32 = 3.2 MB per DMA)
NT = C // F          # 16 tiles

S = 64.0
MARGIN = 0.5
EPS = 1e-7

_cached_nc = None


def _build():
    nc = bacc.Bacc(
        "TRN2",
        target_bir_lowering=False,
        debug=False,
        num_devices=N_CORES,
    )
    pred = nc.dram_tensor("pred", [ROWS, C], mybir.dt.float32, kind="ExternalInput").ap()
    out = nc.dram_tensor("out", [ROWS, 1], mybir.dt.float32, kind="ExternalOutput").ap()

    with tile.TileContext(nc) as tc:
        with (
            tc.tile_pool(name="inp", bufs=4) as inp_pool,
            tc.tile_pool(name="scr", bufs=2) as scr_pool,
            tc.tile_pool(name="acc", bufs=1) as acc_pool,
        ):
            partials = acc_pool.tile([ROWS, NT], mybir.dt.float32)
            for t in range(NT):
                tin = inp_pool.tile([ROWS, F], mybir.dt.float32)
                nc.sync.dma_start(tin[:], pred[:, bass.ts(t, F)])
                texp = scr_pool.tile([ROWS, F], mybir.dt.float32)
                # texp = exp(64 * tin); partials[:, t] = row-sum(texp), fused.
                nc.scalar.activation(
                    texp[:],
                    tin[:],
                    mybir.ActivationFunctionType.Exp,
                    scale=S,
                    accum_out=partials[:, t : t + 1],
                )
            rowsum = acc_pool.tile([ROWS, 1], mybir.dt.float32)
            nc.vector.reduce_sum(rowsum[:], partials[:], axis=mybir.AxisListType.X)
            nc.sync.dma_start(out[:], rowsum[:])

    nc.compile()
    return nc


def _get_nc():
    global _cached_nc
    if _cached_nc is None:
        _cached_nc = _build()
    return _cached_nc


def _device_row_sums(pred: np.ndarray, trace: bool = False):
    """Run the SPMD kernel; returns (row_sum[1024] f64, BassKernelResults)."""
    nc = _get_nc()
    in_maps = [{"pred": pred[c * ROWS : (c + 1) * ROWS]} for c in range(N_CORES)]
    res = run_bass_kernel_spmd(nc, in_maps, core_ids=list(range(N_CORES)), trace=trace)
    row_sum = np.concatenate(
        [res.results[c]["out"][:, 0] for c in range(N_CORES)]
    ).astype(np.float64)
    return row_sum, res


def kernel(pred: np.ndarray, labels: np.ndarray) -> np.ndarray:
    pred = np.ascontiguousarray(pred, dtype=np.float32)
    labels = np.asarray(labels).astype(np.int64)
    assert pred.shape == (B, C) and labels.shape == (B,)

    row_sum, _ = _device_row_sums(pred)

    tgt = pred[np.arange(B), labels].astype(np.float64)
    tclip = np.clip(tgt, -1.0 + EPS, 1.0 - EPS)
    numerator = S * np.cos(np.arccos(tclip) + MARGIN)
    excl = row_sum - np.exp(S * tgt)
    denom = np.exp(numerator) + excl
    loss = -np.mean(numerator - np.log(denom))
    return np.asarray(loss, dtype=np.float32)


# revision 2
# speedup vs baseline: 1.1983x; 1.1983x over previous
"""ArcFace (AngularPenaltySMLoss) on 8 TRN2 NeuronCores.

Strategy: data-parallel over batch rows. pred is [1024, 100000] f32; each of
the 8 cores gets a [128, 100000] shard and computes, per row, the full-row
sum of exp(64 * pred) in a single streaming pass: HWDGE DMA loads column
tiles into SBUF while the ScalarEngine runs activation(Exp, scale=64) with
the fused per-partition accumulator (accum_out), one partial per tile.
Raw Bass (no Tile framework) keeps the prologue/epilogue overhead minimal;
tile widths taper at the end so the last activation barely trails the last
DMA. The tiny epilogue (label gather, arccos/cos numerator, log, mean) is
O(B) and runs on host.
"""

import sys
from contextlib import ExitStack

import numpy as np

_REPO = "/opt/trn_rl_repo"
if _REPO not in sys.path:
    sys.path.insert(0, _REPO)

import concourse.bass as bass
from concourse import mybir
from concourse.bass_utils import run_bass_kernel_spmd

B, C = 1024, 100000
N_CORES = 8
ROWS = B // N_CORES  # 128 rows per core = SBUF partition count

# Column-tile widths: big steady-state tiles (6.4 MB DMAs), tapering at the
# end so the final activation+reduce tail after the last DMA is short.
WIDTHS = [12500] * 7 + [6250, 3125, 1875, 1250]
assert sum(WIDTHS) == C
NT = len(WIDTHS)
WMAX = max(WIDTHS)
NB = 2  # input double-buffer

S = 64.0
MARGIN = 0.5
EPS = 1e-7

_cached_nc = None


def _build():
    nc = bass.Bass(
        "TRN2",
        target_bir_lowering=False,
        debug=False,
        num_devices=N_CORES,
    )
    pred = nc.dram_tensor("pred", [ROWS, C], mybir.dt.float32, kind="ExternalInput").ap()
    out = nc.dram_tensor("out", [ROWS, NT], mybir.dt.float32, kind="ExternalOutput").ap()

    with ExitStack() as ctx:
        bufs = [
            ctx.enter_context(nc.sbuf_tensor(f"in{i}", [ROWS, WMAX], mybir.dt.float32))
            for i in range(NB)
        ]
        scratch = ctx.enter_context(
            nc.sbuf_tensor("scratch", [ROWS, WMAX], mybir.dt.float32)
        )
        partials = ctx.enter_context(
            nc.sbuf_tensor("partials", [ROWS, NT], mybir.dt.float32)
        )
        probe = ctx.enter_context(nc.sbuf_tensor("probe", [ROWS, NT], mybir.dt.float32))
        dma_sem = ctx.enter_context(nc.semaphore("dma_sem"))
        act_sem = ctx.enter_context(nc.semaphore("act_sem"))
        done_sem = ctx.enter_context(nc.semaphore("done_sem"))
        block = ctx.enter_context(nc.Block(no_gpsimd_drain=True))

        offs = np.cumsum([0] + WIDTHS).tolist()

        @block.sync
        def _(sync):
            for t, w in enumerate(WIDTHS):
                if t >= NB:
                    # WAR: the activation of tile t-NB must have consumed
                    # this buffer before we overwrite it.
                    sync.wait_ge(act_sem, t - NB + 1)
                sync.dma_start(
                    bufs[t % NB][:, :w], pred[:, offs[t] : offs[t] + w]
                ).then_inc(dma_sem, 16)
            sync.wait_ge(done_sem, 1)
            sync.dma_start(out[:], probe[:]).then_inc(dma_sem, 16)
            sync.wait_ge(dma_sem, 16 * (NT + 1))

        @block.scalar
        def _(scalar):
            for t, w in enumerate(WIDTHS):
                scalar.wait_ge(dma_sem, 16 * (t + 1))
                scalar.activation(
                    scratch[:, :w],
                    bufs[t % NB][:, :w],
                    mybir.ActivationFunctionType.Exp,
                    scale=S,
                    accum_out=partials[:, t : t + 1],
                ).then_inc(act_sem, 1)
            # Same-engine program order guarantees every accumulator read
            # has retired; this copy's completion makes the partials safe
            # to DMA (probe holds the finished values).
            scalar.copy(probe[:], partials[:]).then_inc(done_sem, 1)

    return nc


def _get_nc():
    global _cached_nc
    if _cached_nc is None:
        _cached_nc = _build()
    return _cached_nc


def _device_row_sums(pred: np.ndarray, trace: bool = False):
    """Run the SPMD kernel; returns (row_sum[1024] f64, BassKernelResults)."""
    nc = _get_nc()
    in_maps = [{"pred": pred[c * ROWS : (c + 1) * ROWS]} for c in range(N_CORES)]
    res = run_bass_kernel_spmd(nc, in_maps, core_ids=list(range(N_CORES)), trace=trace)
    partials = np.concatenate(
        [res.results[c]["out"] for c in range(N_CORES)], axis=0
    ).astype(np.float64)
    row_sum = partials.sum(axis=1)
    return row_sum, res


def kernel(pred: np.ndarray, labels: np.ndarray) -> np.ndarray:
    pred = np.ascontiguousarray(pred, dtype=np.float32)
    labels = np.asarray(labels).astype(np.int64)
    assert pred.shape == (B, C) and labels.shape == (B,)

    row_sum, _ = _device_row_sums(pred)

    tgt = pred[np.arange(B), labels].astype(np.float64)
    tclip = np.clip(tgt, -1.0 + EPS, 1.0 - EPS)
    numerator = S * np.cos(np.arccos(tclip) + MARGIN)
    excl = row_sum - np.exp(S * tgt)
    denom = np.exp(numerator) + excl
    loss = -np.mean(numerator - np.log(denom))
    return np.asarray(loss, dtype=np.float32)
